# revision 2
# baseline (speedup 1.0000x reference)
"""HashEncoding (multires hash grid, 16 levels, F=2) for 8 trn2 NeuronCores.

Data-parallel: 2^20 points sharded 8 ways (131072/core = 128 partitions x
1024), hash table (64 MiB) replicated. Per (level, chunk) the DVE computes
floor/ceil coords, the 19-bit XOR hash for the 8 cell corners (multiplies
split so every int product stays fp32-exact), and trilinear lerp weights.
The 8x256 corner rows are fetched with back-to-back GPSIMD indirect DMAs
(one [128,1]-index instruction per point-column per corner -- the only
form the TRN2 SWDGE supports; ~1.6us each is the hard per-instruction
floor measured on HW). Emission is software-pipelined one level ahead so
the Pool engine never waits on index computation, and blends/lerps run on
the DVE under the gather stream. Output staged in SBUF, one contiguous
DMA per chunk.
"""
import sys

sys.path.insert(0, "/opt/trn_rl_repo")

import numpy as np

NUM_LEVELS = 16
F = 2
TABLE_SIZE = 1 << 19
MASK = TABLE_SIZE - 1
N_POINTS = 1 << 20
N_CORES = 8
PPC = 1024          # points per partition (131072 per core / 128)
CH = 256            # points per partition per chunk
SCALINGS = [16.0, 23.0, 33.0, 48.0, 70.0, 101.0, 147.0, 212.0,
            307.0, 445.0, 645.0, 933.0, 1351.0, 1955.0, 2830.0, 4095.0]
P1_19 = 2654435761 % TABLE_SIZE   # 489905
P2_19 = 805459861 % TABLE_SIZE    # 153493

_CACHE = {}


def build_program(ppc=PPC, ch=CH):
    import concourse.bass as bass
    import concourse.tile as tile
    from concourse import bacc, mybir

    F32 = mybir.dt.float32
    I32 = mybir.dt.int32
    npts = 128 * ppc
    nch = ppc // ch

    AND = mybir.AluOpType.bitwise_and
    OR = mybir.AluOpType.bitwise_or
    XOR = mybir.AluOpType.bitwise_xor
    MUL = mybir.AluOpType.mult
    ADD = mybir.AluOpType.add
    GT = mybir.AluOpType.is_gt
    NE = mybir.AluOpType.not_equal
    SHR = mybir.AluOpType.logical_shift_right

    nc = bacc.Bacc("TRN2", target_bir_lowering=False, debug=False,
                   num_devices=N_CORES)
    xs = nc.dram_tensor("xs", [npts, 3], F32, kind="ExternalInput").ap()
    tbl = nc.dram_tensor("tbl", [TABLE_SIZE * NUM_LEVELS, F], F32,
                         kind="ExternalInput").ap()
    enc = nc.dram_tensor("enc", [npts, NUM_LEVELS * F], F32,
                         kind="ExternalOutput").ap()

    xr = xs.rearrange("(p k) d -> p (k d)", p=128)
    enc_r = enc.rearrange("(p k) f -> p k f", p=128)   # [128, ppc, 32]

    with tile.TileContext(nc) as tc:
        with (
            tc.tile_pool(name="xpool", bufs=1) as xpool,
            tc.tile_pool(name="scr", bufs=1) as scr,
            tc.tile_pool(name="lvl", bufs=1) as lvlp,
            tc.tile_pool(name="gp", bufs=1) as gp,
            tc.tile_pool(name="bl", bufs=1) as bl,
            tc.tile_pool(name="accp", bufs=2) as accp,
        ):
            xt = xpool.tile([128, ppc * 3], F32)
            nc.sync.dma_start(xt[:], xr)
            x3 = xt.rearrange("p (k d) -> p k d", d=3)
            xd = []
            for d in range(3):
                t = xpool.tile([128, ppc], F32, tag=f"xd{d}")
                nc.vector.tensor_copy(t[:], x3[:, :, d])
                xd.append(t)

            with tc.For_i(0, nch) as i:
                acc = accp.tile([128, ch, NUM_LEVELS * F], F32, tag="acc")

                def prep(lvl):
                    """DVE: coords, hashes, corner index tiles for level."""
                    S = SCALINGS[lvl]
                    OFF = lvl << 19
                    pp = lvl % 2
                    od, fl_i, ne_i = [], [], []
                    for d in range(3):
                        s = scr.tile([128, ch], F32, tag=f"s{d}")
                        nc.vector.tensor_scalar(
                            s[:], xd[d][:, bass.ts(i, ch)], S, None, MUL)
                        ri = scr.tile([128, ch], I32, tag=f"ri{d}")
                        nc.vector.tensor_copy(ri[:], s[:])   # round-nearest
                        rf = scr.tile([128, ch], F32, tag=f"rf{d}")
                        nc.vector.tensor_copy(rf[:], ri[:])
                        gt = scr.tile([128, ch], F32, tag=f"gt{d}")
                        nc.vector.tensor_tensor(gt[:], rf[:], s[:], GT)
                        ff = scr.tile([128, ch], F32, tag=f"ff{d}")
                        nc.vector.tensor_sub(ff[:], rf[:], gt[:])   # floor
                        o = lvlp.tile([128, ch], F32, tag=f"od{d}_{pp}")
                        nc.vector.tensor_sub(o[:], s[:], ff[:])
                        ne = scr.tile([128, ch], F32, tag=f"ne{d}")
                        nc.vector.tensor_tensor(ne[:], ff[:], s[:], NE)
                        fi = scr.tile([128, ch], I32, tag=f"fi{d}")
                        nc.vector.tensor_copy(fi[:], ff[:])
                        nei = scr.tile([128, ch], I32, tag=f"nei{d}")
                        nc.vector.tensor_copy(nei[:], ne[:])
                        od.append(o); fl_i.append(fi); ne_i.append(nei)

                    ci = scr.tile([128, ch], I32, tag="ci")
                    nc.vector.tensor_add(ci[:], fl_i[0][:], ne_i[0][:])

                    def hash_fc(d, prime, tag):
                        q1 = (prime * 32) & MASK
                        q2 = (prime * 1024) & MASK
                        fi, nei = fl_i[d], ne_i[d]
                        y0 = scr.tile([128, ch], I32, tag=tag + "y0")
                        nc.vector.tensor_scalar(y0[:], fi[:], 31, None, AND)
                        y1 = scr.tile([128, ch], I32, tag=tag + "y1")
                        nc.vector.tensor_scalar(y1[:], fi[:], 5, 31, SHR, AND)
                        y2 = scr.tile([128, ch], I32, tag=tag + "y2")
                        nc.vector.tensor_scalar(y2[:], fi[:], 10, None, SHR)
                        m0 = scr.tile([128, ch], I32, tag=tag + "m0")
                        nc.vector.tensor_scalar(m0[:], y0[:], prime, None, MUL)
                        nc.vector.tensor_scalar(m0[:], m0[:], MASK, None, AND)
                        m1 = scr.tile([128, ch], I32, tag=tag + "m1")
                        nc.vector.tensor_scalar(m1[:], y1[:], q1, None, MUL)
                        nc.vector.tensor_scalar(m1[:], m1[:], MASK, None, AND)
                        m2 = scr.tile([128, ch], I32, tag=tag + "m2")
                        nc.vector.tensor_scalar(m2[:], y2[:], q2, None, MUL)
                        h = scr.tile([128, ch], I32, tag=tag + "h")
                        nc.vector.tensor_tensor(h[:], m0[:], m1[:], ADD)
                        nc.vector.tensor_tensor(h[:], h[:], m2[:], ADD)
                        hf = scr.tile([128, ch], I32, tag=tag + "hf")
                        nc.vector.tensor_scalar(hf[:], h[:], MASK, None, AND)
                        hc = scr.tile([128, ch], I32, tag=tag + "hc")
                        nc.vector.scalar_tensor_tensor(
                            hc[:], nei[:], prime, hf[:], MUL, ADD)
                        nc.vector.tensor_scalar(hc[:], hc[:], MASK, None, AND)
                        return hf, hc

                    h1f, h1c = hash_fc(1, P1_19, "hy")
                    h2f, h2c = hash_fc(2, P2_19, "hz")

                    def txor(a, b, tag):
                        t = scr.tile([128, ch], I32, tag=tag)
                        nc.vector.tensor_tensor(t[:], a[:], b[:], XOR)
                        return t

                    t_cc = txor(h1c, h2c, "tcc")
                    t_fc = txor(h1f, h2c, "tfc")
                    t_cf = txor(h1c, h2f, "tcf")
                    t_ff = txor(h1f, h2f, "tff")

                    xf = fl_i[0]
                    corner_spec = [(ci, t_cc), (ci, t_fc), (xf, t_fc),
                                   (xf, t_cc), (ci, t_cf), (ci, t_ff),
                                   (xf, t_ff), (xf, t_cf)]
                    idxs = []
                    for c, (xp, tp) in enumerate(corner_spec):
                        rc = scr.tile([128, ch], I32, tag="rc")
                        nc.vector.tensor_tensor(rc[:], xp[:], tp[:], XOR)
                        ix = lvlp.tile([128, ch], I32, tag=f"ix{c}_{pp}")
                        nc.vector.tensor_scalar(ix[:], rc[:], OFF, None, OR)
                        idxs.append(ix)
                    return idxs, od

                def gathers(lvl, idxs):
                    pp = lvl % 2
                    gs = []
                    for c in range(8):
                        g = gp.tile([128, ch, F], F32, tag=f"g{c}_{pp}")
                        gs.append(g)
                    for c in range(8):
                        for k in range(ch):
                            nc.gpsimd.indirect_dma_start(
                                out=gs[c][:, k, :],
                                out_offset=None,
                                in_=tbl[:],
                                in_offset=bass.IndirectOffsetOnAxis(
                                    ap=idxs[c][:, k:k + 1], axis=0),
                            )
                    return gs

                def blends(lvl, gs, od):
                    oxb = od[0][:].to_broadcast([128, ch, F])
                    oyb = od[1][:].to_broadcast([128, ch, F])
                    ozb = od[2][:].to_broadcast([128, ch, F])

                    def lerp(ga, gb, wb, outap):
                        t0 = bl.tile([128, ch, F], F32, tag="t0")
                        nc.vector.tensor_sub(t0[:], ga, gb)
                        nc.vector.tensor_mul(t0[:], t0[:], wb)
                        nc.vector.tensor_add(outap, t0[:], gb)

                    def btile(tag):
                        bt = bl.tile([128, ch, F], F32, tag=tag)
                        return bt

                    l03 = btile("l03"); lerp(gs[0][:], gs[3][:], oxb, l03[:])
                    l12 = btile("l12"); lerp(gs[1][:], gs[2][:], oxb, l12[:])
                    l47 = btile("l47"); lerp(gs[4][:], gs[7][:], oxb, l47[:])
                    l56 = btile("l56"); lerp(gs[5][:], gs[6][:], oxb, l56[:])
                    ly1 = btile("ly1"); lerp(l03[:], l12[:], oyb, ly1[:])
                    ly0 = btile("ly0"); lerp(l47[:], l56[:], oyb, ly0[:])
                    accsl = acc[:, :, lvl * F:(lvl + 1) * F]
                    lerp(ly1[:], ly0[:], ozb, accsl)

                state = {}
                state[0] = prep(0)
                for lvl in range(NUM_LEVELS):
                    if lvl + 1 < NUM_LEVELS:
                        state[lvl + 1] = prep(lvl + 1)
                    idxs, od = state.pop(lvl)
                    gs = gathers(lvl, idxs)
                    blends(lvl, gs, od)

                nc.sync.dma_start(enc_r[:, bass.ts(i, ch), :], acc[:])
    nc.compile()
    return nc


def _get_program():
    key = (PPC, CH)
    if key not in _CACHE:
        _CACHE[key] = build_program()
    return _CACHE[key]


def kernel(x: np.ndarray, hash_table: np.ndarray) -> np.ndarray:
    from concourse.bass_utils import run_bass_kernel_spmd

    nc = _get_program()
    x = np.ascontiguousarray(np.asarray(x, dtype=np.float32))
    tb = np.ascontiguousarray(np.asarray(hash_table, dtype=np.float32))
    npc = N_POINTS // N_CORES
    in_maps = [
        {"xs": x[c * npc:(c + 1) * npc], "tbl": tb} for c in range(N_CORES)
    ]
    res = run_bass_kernel_spmd(nc, in_maps, list(range(N_CORES)))
    return np.concatenate(
        [res.results[c]["enc"] for c in range(N_CORES)], axis=0)
